# revision 68
# speedup vs baseline: 1.0496x; 1.0013x over previous
"""Trainium2 Bass kernel for nn_Attention_55293408968939.

Full-input contract: kernel(**inputs) takes the unsharded inputs and returns
the full [1, 2048, 2048] output. Internally: 16 heads are sharded 2-per-core
across 8 NeuronCores (tensor parallel); each core computes QKV projection for
its heads, RMSNorm+3D-RoPE, non-causal attention, and its partial output
projection; the host sums the 8 partials and adds the (folded) bias row.

v3 highlights over the fp32r baseline:
  - bf16 value path everywhere (x, weights, tables, q/k/v, E, ctx, output
    partial): halves DMA traffic, enables DVE 2x perf modes; PE cost is
    unchanged (bf16 matmul = 1 cycle/row, same as fp32r).
  - softmax row sums via fp8e4 DoubleRow matmuls (0.5 cycles/row): the exp
    writes E in bf16 for the PV matmul, a Pool SWDGE cast DMA produces an
    fp8 copy, and 8 DoubleRow ones-matmuls (2 k-tiles each) accumulate the
    denominator. The sums tolerate fp8 easily (positive summands add
    coherently, quantization errors cancel). Softmax is computed shifted
    (exp(s - C_SHIFT)), which cancels in the normalization and keeps the
    fp8 copies in range.
  - RMS factor for K (with the full 1/sqrt(D) attention scale) is applied
    as the exp's per-partition scale; the ln(mean sq) rows are transposed
    to columns by a small DRAM-bounce DMA per chunk, off the critical path.
    Only Q takes the broadcast-multiply path.
  - a deferred-work queue feeds the in-order PE: each block's softmax-sum
    matmuls (which wait on the cast DMA) and the previous q-chunk's proj
    units are emitted into later blocks' free slots, so the PE never
    stalls on them.
  - DMA batching/routing: weight/x loads are multi-tile chunks with small
    leading pieces; x goes through the Pool SWDGE path so HWDGE serves only
    weights and output drains; the last q chunk runs as two half-width
    passes and the final proj units ship via per-row buffers on both DMA
    paths to shorten the tail.

v4: the q c4=3 projections move out of phase 1 into phase 2's deferred
  queue as PE filler for the first (ACT-exp-paced) attention blocks. Their
  x chunk stays SBUF-resident, loaded as ONE contiguous DMA on the idle
  weight queue, explicitly dependency-gated on c4=2's first matmul so the
  prefetch cannot steal DMA bandwidth from earlier chunk feeds (eager
  prefetch measured +13us). The burst accumulators borrow the ps_o proj
  banks, which are idle until the first proj unit (~slot 33) -- crucially
  ps_st keeps 3 bufs (cutting to 2 costs +9.1us of S-lookahead). The
  deferred queue is a heap (bursts seed ahead of later-ready entries);
  the E8 buffer is split into two half-size tiles to pay for the resident
  x chunk in SBUF.

Per-core dataflow (all matmuls bf16 with fp32 PSUM accumulate):
  phase 1: qT/kT computed transposed [head_dim, tok] straight from the matmul
           (lhsT = w chunk, rhs = xT chunk); v computed natural [tok, head_dim]
           (lhsT = xT chunk, rhs = wvT chunk). q RMS factor r_q applied via
           GPSIMD partition broadcast + DVE multiply after RoPE; k RMS factor
           deferred to phase 2. RoPE uses host-folded cos/sin tables (norm
           weight + pair signs folded in) with a quadrant-local de-interleave
           so the pair swap is a stream_shuffle (+-16 per 32-partition block).
  phase 2: per (head, 512-token q chunk): ST[k,q] = kT.T-tile @ qT (16 k
           tiles); E = exp(r_k * ST - C_SHIFT) on ACT with r_k[128,1] as the
           per-partition activation scale; softmax sums via ones-matmul
           accumulation; PV via lhsT = v tile accumulation -> ctxT [d, q];
           normalize by a DVE Newton-Raphson reciprocal of the sums,
           GPSIMD-broadcast. Proj units for the previous q chunk interleave.
  phase 3: partial = ctxT.T @ proj_wT slice, drained and DMA'd out (bf16).

Host folds: qkv v-bias contributes exactly bias_v @ proj_w.T to the output
(softmax rows sum to 1), so it is added host-side with proj_b.
"""
import heapq
import sys

sys.path.insert(0, "/opt/trn_rl_repo")

import numpy as np
import ml_dtypes

NUM_HEADS = 16
N_CORES = 8
D = 128           # head dim
N = 2048          # tokens
C = 2048          # model dim
EPS = 1e-6
ROPE_THETA = 10000.0
C_SHIFT = 1.5     # softmax shift: exp(s - C_SHIFT); cancels in normalization

_CACHE = {}

BF = ml_dtypes.bfloat16


def _perm_quadrant():
    """Partition permutation: quadrant b lanes 0-15 = even dims of [32b,32b+32),
    lanes 16-31 = odd dims. perm[p] = original head-dim index stored at lane p."""
    perm = np.empty(128, np.int64)
    for b in range(4):
        for j in range(16):
            perm[32 * b + j] = 32 * b + 2 * j
            perm[32 * b + 16 + j] = 32 * b + 2 * j + 1
    return perm


def _rope_tables(T, H, W, head_dim):
    dh = 2 * ((head_dim // 3) // 2)
    dw = dh
    dt = head_dim - dh - dw

    def axis_ang(L, d):
        inv = 1.0 / (ROPE_THETA ** (np.arange(0, d, 2, dtype=np.float32) / d))
        return np.arange(L, dtype=np.float32)[:, None] * inv[None, :]

    at = axis_ang(T, dt)
    ah = axis_ang(H, dh)
    aw = axis_ang(W, dw)
    at_g = np.broadcast_to(at[:, None, None, :], (T, H, W, dt // 2))
    ah_g = np.broadcast_to(ah[None, :, None, :], (T, H, W, dh // 2))
    aw_g = np.broadcast_to(aw[None, None, :, :], (T, H, W, dw // 2))
    ang = np.concatenate([at_g, ah_g, aw_g], axis=-1).reshape(T * H * W, head_dim // 2)
    return np.cos(ang), np.sin(ang)  # [N, 64] fp32


def _folded_tables(cos, sin, w, perm):
    """cosT/sinT [128, N] in the quadrant-deinterleaved transposed layout with
    norm weight and rotation signs folded in.

    lane p holds dim d = perm[p], pair index i = d // 2.
    m1 coeff at lane p = cos_i * w[d].
    After the +-16 quadrant shuffle, lane p holds the partner dim value, so
    m2 coeff = -sin_i * w[d+1] for even d, +sin_i * w[d-1] for odd d."""
    n = cos.shape[0]
    cosT = np.empty((128, n), np.float32)
    sinT = np.empty((128, n), np.float32)
    for p in range(128):
        d = int(perm[p])
        i = d // 2
        cosT[p] = cos[:, i] * w[d]
        if d % 2 == 0:
            sinT[p] = -sin[:, i] * w[d + 1]
        else:
            sinT[p] = sin[:, i] * w[d - 1]
    return cosT, sinT


def _build_nc(debug=False):
    import concourse.bacc as bacc
    import concourse.bass_isa as bass_isa
    import concourse.mybir as mybir
    import concourse.tile as tile

    F32 = mybir.dt.float32
    BF16 = mybir.dt.bfloat16
    F8 = mybir.dt.float8e4
    AF = mybir.ActivationFunctionType
    SHUF_MASK = list(range(16, 32)) + list(range(0, 16))

    # Restrict ACT table-set choice to natural_log_exp_and_others (covers
    # Identity/Copy/Ln/Exp) so the whole kernel needs ONE table load instead
    # of alternating set loads (~1.3us each).
    _orig_tables = bacc.get_activation_tables

    def _one_set(arch):
        tabs = _orig_tables(arch)
        return {nm: (s if nm == "natural_log_exp_and_others" else set())
                for nm, s in tabs.items()}

    bacc.get_activation_tables = _one_set

    nc = bacc.Bacc("TRN2", target_bir_lowering=False, debug=False,
                   num_devices=N_CORES)

    # ---- DRAM I/O ----
    xT_d = nc.dram_tensor("xT", [C, N], BF16, kind="ExternalInput")
    wqk_d = nc.dram_tensor("wqkT", [C, 512], BF16, kind="ExternalInput")
    wv_d = nc.dram_tensor("wvT", [C, 256], BF16, kind="ExternalInput")
    pw_d = nc.dram_tensor("projwT", [256, C], BF16, kind="ExternalInput")
    bqk_d = nc.dram_tensor("bias_qk", [128, 4], F32, kind="ExternalInput")
    cq_d = nc.dram_tensor("cosq", [128, N], BF16, kind="ExternalInput")
    sq_d = nc.dram_tensor("sinq", [128, N], BF16, kind="ExternalInput")
    ck_d = nc.dram_tensor("cosk", [128, N], BF16, kind="ExternalInput")
    sk_d = nc.dram_tensor("sink", [128, N], BF16, kind="ExternalInput")
    ones_d = nc.dram_tensor("ones", [128, 1], BF16, kind="ExternalInput")
    eps_d = nc.dram_tensor("epsc", [1, 1], F32, kind="ExternalInput")
    out_d = nc.dram_tensor("partial", [N, C], BF16, kind="ExternalOutput")
    lnkb_d = nc.dram_tensor("lnk_bounce", [2, N], F32, kind="Internal")
    if debug:
        dbg_qk = [nc.dram_tensor(f"dbg_qk{i}", [128, N], BF16, kind="ExternalOutput")
                  for i in range(4)]
        dbg_v = nc.dram_tensor("dbg_v", [128, 16, 256], BF16, kind="ExternalOutput")
        dbg_rk = nc.dram_tensor("dbg_rk", [128, 32], F32, kind="ExternalOutput")
        dbg_ctx = nc.dram_tensor("dbg_ctx", [128, 2, N], BF16, kind="ExternalOutput")

    tab_dram = {"cq": cq_d, "sq": sq_d, "ck": ck_d, "sk": sk_d}

    with tile.TileContext(nc) as tc:
        with (
            tc.tile_pool(name="persist", bufs=1) as pp,
            tc.tile_pool(name="rows", bufs=2) as rows,
        ):
            # resident SBUF tensors
            wqk_sb = pp.tile([128, 16, 512], BF16, name="wqk_sb")
            wv_sb = pp.tile([128, 16, 256], BF16, name="wv_sb")
            pw_sb = pp.tile([128, 2, C], BF16, name="pw_sb")
            tab_sb = {nm: pp.tile([128, N], BF16, name=f"tab_{nm}")
                      for nm in ("cq", "sq", "ck", "sk")}
            bqk_sb = pp.tile([128, 4], F32, name="bqk_sb")
            ones_sb = pp.tile([128, 1], BF16, name="ones_sb")
            nc.vector.memset(ones_sb[:], 1.0)
            # dual-fp8 ldweights needs a 16B-aligned column stride
            ones8_sb = pp.tile([128, 2, 16], F8, name="ones8_sb")
            nc.vector.memset(ones8_sb[:], 1.0)
            eps_sb = pp.tile([1, 1], F32, name="eps_sb")
            nc.vector.memset(eps_sb[:], float(EPS))

            # x chunk 3 stays resident so its q projections can run as
            # deferred PE filler during phase 2's first (ACT-paced) blocks
            xt3_sb = pp.tile([128, 16, 512], BF16, name="xt3_sb")
            # final q/k (transposed, rope'd; q scaled by r_q) and v, ctx
            qk_f = [pp.tile([128, N], BF16, name=f"qkf{i}") for i in range(4)]
            v_sb = pp.tile([128, 16, 256], BF16, name="v_sb")
            ctx_sb = pp.tile([128, 2, N], BF16, name="ctx_sb")
            # ln(mean k^2 + eps) rows per k head, and the transposed r_k cols
            lnk_sb = [pp.tile([1, N], F32, name=f"lnk{h}") for h in range(2)]
            rkcol_sb = [pp.tile([128, 16], F32, name=f"rkc{h}") for h in range(2)]
            rk_sb = [pp.tile([128, 16], F32, name=f"rk{h}") for h in range(2)]
            negc_sb = pp.tile([128, 1], F32, name="negc_sb")
            nc.vector.memset(negc_sb[:], -float(C_SHIFT))
            logd_sb = pp.tile([128, 1], F32, name="logd_sb")
            nc.vector.memset(logd_sb[:], float(-0.5 * np.log(float(D))))
            zero_sb = pp.tile([1, 1], F32, name="zero_sb")
            nc.vector.memset(zero_sb[:], 0.0)

            # table per tensor index: 0:q0 1:k0 2:q1 3:k1
            tab_of = [("cq", "sq"), ("ck", "sk"), ("cq", "sq"), ("ck", "sk")]

            # ---------------- phase 1: QKV + RMS + RoPE ----------------
            with (
                tc.tile_pool(name="xt", bufs=3) as xtp,
                tc.tile_pool(name="qraw", bufs=6) as qrawp,
                tc.tile_pool(name="scr", bufs=2) as scr,
                tc.tile_pool(name="rbc", bufs=4) as rbcp,
                tc.tile_pool(name="redp", bufs=2) as redp,
                tc.tile_pool(name="ep", bufs=2) as ep,
                tc.tile_pool(name="e8p", bufs=2) as e8p,
                tc.tile_pool(name="invb", bufs=1) as invbp,
                tc.tile_pool(name="outp", bufs=6) as outp,
            ):
                rbcs_of = {}

                def rope_A(c4, only_f=None):
                    tsl = slice(c4 * 512, (c4 + 1) * 512)
                    rbcs = rbcs_of.setdefault(c4, {})
                    for f in ((1, 3, 0, 2) if only_f is None else (only_f,)):
                        qraw = qraw_tiles[(c4, f)]
                        sq = scr.tile([128, 512], BF16, tag="sq", name=f"sq{c4}_{f}")
                        nc.vector.tensor_mul(sq[:], qraw[:], qraw[:])
                        ssq = redp.tile([128, 512], F32, tag="red", name=f"ssq{c4}_{f}")
                        nc.gpsimd.partition_all_reduce(ssq[:], sq[:], 128,
                                                       bass_isa.ReduceOp.add)
                        if f in (1, 3):
                            # k: ln(mean sq + eps) row, transposed to columns
                            # by a small DMA as each chunk's row lands; r_k is
                            # applied in phase 2 as the exp's per-partition
                            # scale.
                            h = f // 2
                            nc.scalar.activation(lnk_sb[h][0:1, tsl],
                                                 ssq[0:1, :], AF.Ln,
                                                 scale=1.0 / 128.0,
                                                 bias=eps_sb[0:1, 0:1])
                            nc.sync.dma_start(lnkb_d[h:h + 1, tsl],
                                              lnk_sb[h][0:1, tsl])
                            nc.sync.dma_start(
                                rkcol_sb[h][:, c4 * 4:(c4 + 1) * 4],
                                lnkb_d[h:h + 1, tsl].rearrange(
                                    "o (i p) -> (o p) i", p=128))
                        else:
                            lnr = rows.tile([1, 512], F32, tag="row", name=f"lnr{c4}_{f}")
                            nc.scalar.activation(lnr[:], ssq[0:1, :], AF.Ln,
                                                 scale=1.0 / 128.0,
                                                 bias=eps_sb[0:1, 0:1])
                            rrow = rows.tile([1, 512], BF16, tag="rowb", name=f"rrow{c4}_{f}")
                            # r_q = (mean sq)^-1/2
                            nc.scalar.activation(rrow[:], lnr[:], AF.Exp,
                                                 scale=-0.5, bias=zero_sb[0:1, 0:1])
                            rbc = rbcp.tile([128, 512], BF16, tag="rbc", name=f"rbc{c4}_{f}")
                            nc.gpsimd.partition_broadcast(rbc[:], rrow[:])
                            rbcs[f] = rbc

                def rope_B(c4, only_f=None):
                    tsl = slice(c4 * 512, (c4 + 1) * 512)
                    rbcs = rbcs_of.get(c4, {})
                    for f in ((1, 3, 0, 2) if only_f is None else (only_f,)):
                        qraw = qraw_tiles[(c4, f)]
                        cosT = tab_sb[tab_of[f][0]]
                        sinT = tab_sb[tab_of[f][1]]
                        m1 = scr.tile([128, 512], BF16, tag="m1", name=f"m1_{c4}_{f}")
                        nc.vector.tensor_mul(m1[:], qraw[:], cosT[:, tsl])
                        sh = scr.tile([128, 512], BF16, tag="sh", name=f"sh{c4}_{f}")
                        nc.vector.stream_shuffle(sh[:], qraw[:], SHUF_MASK)
                        nc.vector.tensor_mul(sh[:], sh[:], sinT[:, tsl])
                        if f in (1, 3):
                            nc.vector.tensor_add(qk_f[f][:, tsl], m1[:], sh[:])
                        else:
                            nc.vector.tensor_add(m1[:], m1[:], sh[:])
                            nc.vector.tensor_mul(qk_f[f][:, tsl], m1[:], rbcs[f][:])

                def dram_chunk(t, r0, nrow, csl=None):
                    """DRAM rows [r0, r0+nrow*128) as [128, nrow, cols]."""
                    src = t[r0:r0 + nrow * 128, :] if csl is None else t[r0:r0 + nrow * 128, csl]
                    return src.rearrange("(i p) c -> p i c", p=128)

                qraw_tiles = {}
                xdma3 = [None]
                ps1 = tc.tile_pool(name="ps_qk", bufs=4, space="PSUM")
                ps_qk = ps1.__enter__()
                ps2 = tc.tile_pool(name="ps_v", bufs=2, space="PSUM")
                ps_v = ps2.__enter__()
                for c4 in range(4):
                    tsl = slice(c4 * 512, (c4 + 1) * 512)
                    fs = (0, 1, 2, 3) if c4 <= 2 else (1, 3)
                    qk_ps = {f: ps_qk.tile([128, 512], F32, tag="qkps", name=f"qkps{c4}_{f}")
                             for f in fs}
                    # [128,1024] = 2 banks, two 256-wide v regions per bank.
                    v_ps = ps_v.tile([128, 1024], F32, tag="vps", name=f"vps{c4}")
                    # small leading chunks so the first matmul starts early
                    groups = [(0, 1), (1, 1), (2, 2), (4, 2), (6, 2), (8, 4), (12, 4)] \
                        if c4 == 0 else [(0, 4), (4, 4), (8, 4), (12, 4)]
                    for g, (i0, glen) in enumerate(groups):
                        if c4 == 3:
                            xt = xt3_sb
                        else:
                            xt = xtp.tile([128, glen, 512], BF16, tag=f"xt{glen}", name=f"xt{c4}_{g}")
                            # x chunks go through the Pool SWDGE path so they
                            # don't contend with the HWDGE weight loads/drains
                            nc.gpsimd.dma_start(xt[:], dram_chunk(xT_d, i0 * 128, glen, tsl))
                        if c4 == 2 and g == 0:
                            # chunk 3 lands as ONE contiguous DMA on the idle
                            # weight queue, gated on c4=2's compute start so it
                            # cannot steal DMA bandwidth from earlier feeds
                            xdma3[0] = nc.sync.dma_start(
                                xt3_sb[:],
                                dram_chunk(xT_d, 0, 16, slice(1536, 2048)))
                        if c4 == 0:
                            # weight chunks interleaved with the first xt
                            # groups so compute can start immediately
                            nc.sync.dma_start(wqk_sb[:, i0:i0 + glen, :],
                                              dram_chunk(wqk_d, i0 * 128, glen))
                            nc.sync.dma_start(wv_sb[:, i0:i0 + glen, :],
                                              dram_chunk(wv_d, i0 * 128, glen))
                            if g == 6:
                                nc.sync.dma_start(bqk_sb[:], bqk_d[:])
                        elif c4 == 1:
                            if g < 2:
                                for nm in (("cq", "sq") if g == 0 else ("ck", "sk")):
                                    nc.sync.dma_start(tab_sb[nm][:], tab_dram[nm][:])
                            elif g == 2:
                                nc.sync.dma_start(pw_sb[:], dram_chunk(pw_d, 0, 2))
                        for i4 in range(glen):
                            i = i0 + i4
                            xi = i if c4 == 3 else i4
                            for f in fs:
                                mm = nc.tensor.matmul(qk_ps[f][:],
                                                      wqk_sb[:, i, f * 128:(f + 1) * 128],
                                                      xt[:, xi, :], start=(i == 0),
                                                      stop=(i == 15))
                                if c4 == 2 and i == 0 and xdma3[0] is not None:
                                    tile.add_dep_helper(
                                        xdma3[0].ins, mm.ins, sync=True,
                                        reason="xt3 prefetch after c4=2 starts")
                                    xdma3[0] = None
                            for j in range(4):
                                nc.tensor.matmul(v_ps[:, j * 256:(j + 1) * 256],
                                                 xt[:, xi, j * 128:(j + 1) * 128],
                                                 wv_sb[:, i, :],
                                                 start=(i == 0 and j % 2 == 0),
                                                 stop=(i == 15),
                                                 skip_group_check=True)
                        # previous chunk's rope chains, one f per group with a
                        # one-group lag between A (RMS chain) and B (DVE ops)
                        # so B never stalls the in-order DVE on A's Pool/ACT
                        # round trip
                        if c4 >= 1:
                            fA = (1, 3, 0, 2)[g]
                            rope_A(c4 - 1, only_f=fA)
                            if g >= 1:
                                rope_B(c4 - 1, only_f=(1, 3, 0)[g - 1])
                    if c4 >= 1:
                        rope_B(c4 - 1, only_f=2)
                    # drain q/k with bias first (k on ACT: the tail chains
                    # hang off it; q on DVE), then v; rope for the PREVIOUS
                    # chunk overlaps this c4's PE
                    for f in (x for x in (1, 3, 0, 2) if x in fs):
                        qraw = qrawp.tile([128, 512], BF16, tag="qraw", name=f"qraw{c4}_{f}")
                        if f in (1, 3):
                            nc.scalar.activation(qraw[:], qk_ps[f][:],
                                                 AF.Identity,
                                                 bias=bqk_sb[:, f:f + 1],
                                                 scale=1.0)
                        else:
                            nc.vector.tensor_scalar_add(qraw[:], qk_ps[f][:],
                                                        bqk_sb[:, f:f + 1])
                        qraw_tiles[(c4, f)] = qraw
                    # drain v: [tok 128, 256] tiles -> v_sb[:, kt, :]
                    for j in range(4):
                        kt = c4 * 4 + j
                        nc.vector.tensor_copy(v_sb[:, kt, :],
                                              v_ps[:, j * 256:(j + 1) * 256])
                # release phase-1 PSUM (waits only on the qraw/v drains)
                ps2.__exit__(None, None, None)
                ps1.__exit__(None, None, None)
                # tail: only the k chains (they gate phase 2); the q c4=3
                # chunks are only needed at qc=3, so their rope runs inside
                # phase 2.
                rope_A(3, only_f=1)
                rope_B(3, only_f=1)
                rope_A(3, only_f=3)
                rope_B(3, only_f=3)

                # ------------- phase 2+3: attention + fused projection -------------
                with (
                    tc.tile_pool(name="ps_st", bufs=3, space="PSUM") as ps_st,
                    tc.tile_pool(name="ps_ctx", bufs=2, space="PSUM") as ps_ctx,
                    tc.tile_pool(name="ps_ssum", bufs=1, space="PSUM") as ps_ssum,
                    tc.tile_pool(name="ps_o", bufs=2, space="PSUM") as ps_o,
                ):
                    ucount = [0]
                    otb_map = {}

                    def proj_unit(qcp, u, last=False):
                        ucount[0] += 1
                        last = last or ucount[0] > 56
                        mt = 4 * qcp + u // 4
                        oc = u % 4
                        msl = slice(mt * 128, (mt + 1) * 128)
                        osl = slice(oc * 512, (oc + 1) * 512)
                        # final units: 4-deep accumulator rotation by
                        # borrowing the (idle) st-tag buffers
                        if last and ucount[0] % 2 == 0:
                            po = ps_st.tile([128, 512], F32, tag="st", name=f"po{mt}_{oc}")
                        else:
                            po = ps_o.tile([128, 512], F32, tag="po", name=f"po{mt}_{oc}")
                        nc.tensor.matmul(po[:], ctx_sb[:, 0, msl], pw_sb[:, 0, osl],
                                         start=True, stop=False)
                        nc.tensor.matmul(po[:], ctx_sb[:, 1, msl], pw_sb[:, 1, osl],
                                         start=False, stop=True)
                        if last:
                            # copies land in a per-mt row buffer (borrowed
                            # from the E pool) and ship as half-row DMAs on
                            # alternating queues
                            if oc == 0:
                                otb_map[mt] = ep.tile([128, C], BF16, tag="e",
                                                      name=f"otb{mt}")
                            otb = otb_map[mt]
                            if ucount[0] % 2 == 1:
                                nc.scalar.copy(otb[:, osl], po[:])
                            else:
                                nc.vector.tensor_copy(otb[:, osl], po[:])
                            if oc % 2 == 1:
                                hsl = slice((oc - 1) * 512, (oc + 1) * 512)
                                eng = nc.gpsimd if oc == 1 else nc.sync
                                eng.dma_start(out_d[msl, hsl], otb[:, hsl])
                        else:
                            ot = outp.tile([128, 512], BF16, tag="ot", name=f"ot{mt}_{oc}")
                            nc.vector.tensor_copy(ot[:], po[:])
                            nc.sync.dma_start(out_d[msl, osl], ot[:])

                    # global phase-2 slot counter and deferred-work queue:
                    # entries are (ready_slot, closure); closures emit PE work
                    # (ssum matmuls of the previous block, proj units) into
                    # free slots so the in-order PE queue never waits on the
                    # E8 cast DMA or the ctx normalize chain.
                    gslot = [0]
                    pend = []
                    pseq = [0]

                    def pq_push(ready, fn):
                        heapq.heappush(pend, (ready, pseq[0], fn))
                        pseq[0] += 1

                    def drain(limit=2):
                        n = 0
                        while pend and pend[0][0] <= gslot[0] and n < limit:
                            heapq.heappop(pend)[2]()
                            n += 1

                    def attention(q0, qw, h, tag, rk_late=(), fp8=False,
                                  emit_units=None, ssum_late=False,
                                  lnk_emit=False, first=False):
                        """One (head, q-range) attention block. fp8: softmax
                        sums via DoubleRow fp8 matmuls on an E8 copy produced
                        by Pool cast DMAs, deferred into the next block's
                        slots. emit_units: (qcp, [u...]) appended to the queue
                        after this block's normalize is emitted."""
                        qsl = slice(q0, q0 + qw)
                        qT = qk_f[2 * h]
                        kT = qk_f[2 * h + 1]
                        ctx_ps = ps_ctx.tile([128, 512], F32, tag="ctxps", name=f"ctxps{tag}")[:, :qw]
                        ssum = ps_ssum.tile([1, 512], F32, tag="ssum", name=f"ssum{tag}")[:, :qw]
                        E = ep.tile([128, 16, 512], BF16, tag="e", name=f"e{tag}")[:, :, :qw]
                        if fp8:
                            E8a = e8p.tile([128, 8, 512], F8, tag="e8", name=f"e8a{tag}")[:, :, :qw]
                            E8b = e8p.tile([128, 8, 512], F8, tag="e8", name=f"e8b{tag}")[:, :, :qw]
                        for kt in range(16):
                            if kt == 12 and rk_late:
                                nc.scalar.activation(rk_sb[h][:, 12:16],
                                                     rkcol_sb[h][:, 12:16],
                                                     AF.Exp, scale=-0.5,
                                                     bias=logd_sb[:, 0:1])
                                rk_late = ()
                            st = ps_st.tile([128, 512], F32, tag="st", name=f"st{tag}_{kt}")[:, :qw]
                            nc.tensor.matmul(st[:], kT[:, kt * 128:(kt + 1) * 128],
                                             qT[:, qsl], start=True, stop=True)
                            nc.scalar.activation(E[:, kt, :], st[:], AF.Exp,
                                                 scale=rk_sb[h][:, kt:kt + 1],
                                                 bias=negc_sb[:, 0:1])
                            if ssum_late:
                                # last block: burst kt0..7 sums once the
                                # earlier deferred work has drained (kt7),
                                # then inline -- shortest possible tail chain
                                if kt == 6:
                                    for j in range(7):
                                        nc.tensor.matmul(ssum[:], ones_sb[:],
                                                         E[:, j, :],
                                                         start=(j == 0),
                                                         stop=False)
                                elif kt >= 7:
                                    nc.tensor.matmul(ssum[:], ones_sb[:],
                                                     E[:, kt, :], start=False,
                                                     stop=(kt == 15))
                            elif fp8 is None:
                                nc.tensor.matmul(ssum[:], ones_sb[:],
                                                 E[:, kt, :],
                                                 start=(kt == 0),
                                                 stop=(kt == 15))
                            elif not fp8:
                                pq_push(gslot[0] + 1, lambda kt=kt:
                                        nc.tensor.matmul(
                                            ssum[:], ones_sb[:],
                                            E[:, kt, :],
                                            start=(kt == 0),
                                            stop=(kt == 15)))
                            if fp8 or ssum_late:
                                # the block's PVs are exp-gated; defer them
                                # past already-ready work so the stall never
                                # lands on the in-order PE queue
                                pq_push(gslot[0] + 1, lambda kt=kt:
                                        nc.tensor.matmul(
                                            ctx_ps[:],
                                            v_sb[:, kt, h * 128:(h + 1) * 128],
                                            E[:, kt, :],
                                            start=(kt == 0), stop=(kt == 15)))
                            else:
                                nc.tensor.matmul(ctx_ps[:],
                                                 v_sb[:, kt, h * 128:(h + 1) * 128],
                                                 E[:, kt, :],
                                                 start=(kt == 0), stop=(kt == 15))
                            if fp8 and first and kt == 3:
                                # very first block: its exps are ACT-paced, so
                                # its cast (and the dependent fp8 sums) would
                                # land late; ship the first half-cast early
                                nc.gpsimd.dma_start(E8a[:, 0:4, :], E[:, 0:4, :])
                            if fp8 and kt == 7:
                                if first:
                                    nc.gpsimd.dma_start(E8a[:, 4:8, :], E[:, 4:8, :])
                                else:
                                    nc.gpsimd.dma_start(E8a[:, 0:8, :], E[:, 0:8, :])
                            gslot[0] += 1
                            drain()
                        if fp8:
                            nc.gpsimd.dma_start(E8b[:, 0:8, :], E[:, 8:16, :])

                        def normalize():
                            halves = ((slice(0, qw),) if not ssum_late else
                                      (slice(0, qw // 2), slice(qw // 2, qw)))
                            for i, hsl in enumerate(halves):
                                qslh = slice(q0 + hsl.start, q0 + hsl.stop)
                                scr2 = rows.tile([1, 512], F32, tag="row", name=f"scr{tag}_{i}")[:, hsl]
                                inv = rows.tile([1, 512], F32, tag="row", name=f"inv{tag}_{i}")[:, hsl]
                                nc.vector.reciprocal_approx_accurate(inv[:], ssum[:, hsl], scr2[:])
                                invb = invbp.tile([128, 512], F32, tag="invb", name=f"invb{tag}_{i}")[:, hsl]
                                nc.gpsimd.partition_broadcast(invb[:], inv[:])
                                nc.vector.tensor_mul(ctx_sb[:, h, qslh],
                                                     ctx_ps[:, hsl], invb[:])
                            if emit_units is not None:
                                qcp, us = emit_units
                                base = gslot[0]
                                for i, u in enumerate(us):
                                    pq_push(base + 2 + i,
                                            lambda qcp=qcp, u=u: proj_unit(qcp, u))

                        if ssum_late or fp8 is None:
                            normalize()
                        elif fp8:
                            base = gslot[0]
                            d = 11 if q0 < 1024 else 2
                            for j in range(8):
                                rdy = base + (1 + d if q0 < 1024
                                              else (1 + d if j < 4 else j - 1 + d))

                                def ssum_j(j=j):
                                    nc.tensor.matmul(
                                        ssum[:], ones8_sb[:, :, 0:1],
                                        (E8a if j < 4 else E8b)[:, 2 * (j % 4):2 * (j % 4) + 2, :],
                                        start=(j == 0), stop=(j == 7),
                                        perf_mode=mybir.MatmulPerfMode.DoubleRow)
                                    if j == 7:
                                        normalize()
                                pq_push(rdy, ssum_j)
                        else:
                            pq_push(gslot[0] + 1, normalize)

                    # rk = exp(-0.5*lnk - 0.5*ln(D)) = r_k/sqrt(D) per head.
                    # Columns 0:12 (token chunks c4=0..2) transposed during
                    # phase 1; columns 12:16 land mid-tail and are exp'd just
                    # before the kt=12 E-exp needs them (rk_late).
                    for h in range(2):
                        nc.scalar.activation(rk_sb[h][:, 0:12],
                                             rkcol_sb[h][:, 0:12],
                                             AF.Exp, scale=-0.5,
                                             bias=logd_sb[:, 0:1])

                    # q c4=3 projection bursts: pure PE filler for the
                    # ACT-paced qc=0 blocks.  They borrow the ps_o proj
                    # banks, which are idle until the first proj unit pops
                    # at ~slot 33 (their drains finish by slot ~22).
                    def defer_q(f, s0):
                        hold = {}

                        def mk(i0):
                            def run():
                                if i0 == 0:
                                    hold["ps"] = ps_o.tile(
                                        [128, 512], F32, tag="po",
                                        name=f"qpsd3_{f}")
                                for i in range(i0, i0 + 4):
                                    nc.tensor.matmul(
                                        hold["ps"][:],
                                        wqk_sb[:, i, f * 128:(f + 1) * 128],
                                        xt3_sb[:, i, :], start=(i == 0),
                                        stop=(i == 15))
                            return run

                        def drain_q():
                            qraw = qrawp.tile([128, 512], BF16, tag="qraw",
                                              name=f"qrawd3_{f}")
                            nc.vector.tensor_scalar_add(qraw[:], hold["ps"][:],
                                                        bqk_sb[:, f:f + 1])
                            qraw_tiles[(3, f)] = qraw

                        for p in range(4):
                            pq_push(s0 + p, mk(4 * p))
                        pq_push(s0 + 5, drain_q)

                    defer_q(0, 1)
                    defer_q(2, 13)

                    for qc in range(3):
                        for h in range(2):
                            attention(qc * 512, 512, h, f"{h}_{qc}",
                                      rk_late=(h,) if qc == 0 else (),
                                      fp8=True,
                                      lnk_emit=(qc == 0 and h == 0),
                                      first=False,
                                      emit_units=(qc, range(16)) if h == 1 else None)
                            if qc == 0:
                                # q c4=3 rope, deferred: only needed at qc=3
                                f = 0 if h == 0 else 2
                                rope_A(3, only_f=f)
                                rope_B(3, only_f=f)
                    # last q chunk in two half-width passes so its proj units
                    # overlap the second half's attention
                    for hv in range(2):
                        for h in range(2):
                            attention(3 * 512 + hv * 256, 256, h, f"{h}_3{hv}",
                                      fp8=not (hv == 1 and h == 1),
                                      ssum_late=(hv == 1 and h == 1),
                                      emit_units=(3, range(hv * 8, hv * 8 + 8))
                                      if h == 1 else None)
                    # flush remaining deferred work (last ssum groups + final
                    # proj units)
                    while pend:
                        heapq.heappop(pend)[2]()

                    if debug:
                        for i in range(4):
                            nc.sync.dma_start(dbg_qk[i][:], qk_f[i][:])
                        nc.sync.dma_start(dbg_v[:], v_sb[:])
                        nc.sync.dma_start(dbg_rk[:, 0:16], rk_sb[0][:])
                        nc.sync.dma_start(dbg_rk[:, 16:32], rk_sb[1][:])
                        nc.sync.dma_start(dbg_ctx[:], ctx_sb[:])

    try:
        nc.compile()
    finally:
        bacc.get_activation_tables = _orig_tables
    return nc


def _host_prep(x, qkv_w, qkv_b, proj_w, proj_b, q_norm_w, k_norm_w, T, H, W):
    perm = _perm_quadrant()
    cos, sin = _rope_tables(T, H, W, D)
    cosq, sinq = _folded_tables(cos, sin, np.asarray(q_norm_w, np.float32), perm)
    cosk, sink = _folded_tables(cos, sin, np.asarray(k_norm_w, np.float32), perm)

    xT = np.ascontiguousarray(np.asarray(x, np.float32)[0].T).astype(BF)
    qkv_w = np.asarray(qkv_w, np.float32)
    qkv_b = np.asarray(qkv_b, np.float32)
    proj_w = np.asarray(proj_w, np.float32)

    shared = dict(xT=xT, cosq=cosq.astype(BF), sinq=sinq.astype(BF),
                  cosk=cosk.astype(BF), sink=sink.astype(BF),
                  ones=np.ones((128, 1), BF),
                  epsc=np.full((1, 1), EPS, np.float32))
    in_maps = []
    for c in range(N_CORES):
        h0 = 2 * c
        wq = [qkv_w[(h0 + j) * D:(h0 + j + 1) * D][perm] for j in range(2)]
        wk = [qkv_w[C + (h0 + j) * D:C + (h0 + j + 1) * D][perm] for j in range(2)]
        bq = [qkv_b[(h0 + j) * D:(h0 + j + 1) * D][perm] for j in range(2)]
        bk = [qkv_b[C + (h0 + j) * D:C + (h0 + j + 1) * D][perm] for j in range(2)]
        wqkT = np.concatenate([wq[0], wk[0], wq[1], wk[1]], axis=0).T
        bias_qk = np.stack([bq[0], bk[0], bq[1], bk[1]], axis=1)
        wvT = qkv_w[2 * C + h0 * D:2 * C + (h0 + 2) * D].T
        projwT = proj_w[:, h0 * D:(h0 + 2) * D].T
        in_maps.append(dict(shared,
                            wqkT=np.ascontiguousarray(wqkT).astype(BF),
                            wvT=np.ascontiguousarray(wvT).astype(BF),
                            projwT=np.ascontiguousarray(projwT).astype(BF),
                            bias_qk=np.ascontiguousarray(bias_qk)))
    v_bias = qkv_b[2 * C:]
    bias_row = (np.asarray(proj_b, np.float32).astype(np.float64)
                + v_bias.astype(np.float64) @ proj_w.astype(np.float64).T)
    return in_maps, bias_row


def kernel(x, qkv_w, qkv_b, proj_w, proj_b, q_norm_w, k_norm_w,
           t_dim, h_dim, w_dim):
    from concourse import bass_utils

    T, H, W = int(t_dim), int(h_dim), int(w_dim)
    if "nc" not in _CACHE:
        _CACHE["nc"] = _build_nc()
    nc = _CACHE["nc"]

    in_maps, bias_row = _host_prep(x, qkv_w, qkv_b, proj_w, proj_b,
                                   q_norm_w, k_norm_w, T, H, W)
    res = bass_utils.run_bass_kernel_spmd(nc, in_maps,
                                          core_ids=list(range(N_CORES)))
    total = np.zeros((N, C), np.float64)
    for r in res.results:
        total += np.asarray(r["partial"], np.float32)
    out = (total + bias_row[None, :]).astype(np.float32)[None]
    return out

